# revision 1
# baseline (speedup 1.0000x reference)
"""Mistral decoder layer (B=1, S=1024, HID=4096, 32 heads, INTER=11008),
tensor-parallel over 8 trn2 NeuronCores (Megatron style), fp16 storage.

v2 over the fp32r baseline:
  - all matmul operands stored fp16 (same 1 cyc/row PE rate, half the DMA
    and SBUF, 2x DVE rate); PSUM stays fp32; stats/scales fp32
  - weights pre-arranged host-side into the exact slab layouts the kernel
    DMAs, so every weight DMA is contiguous 1-8KB per partition line
  - o-proj weight slabs preloaded (resident) so AllReduce #1 can be chunked
    by SEQUENCE: o-proj chunk c -> AR1(c) overlaps o-proj of chunk c+1 and
    the h2 stats of earlier chunks
  - m (silu*gate product) stays resident in SBUF (no DRAM bounce)
  - AllReduce #2 chunked by hidden rows as before (nch2 chunks)
  - residuals folded into the AllReduces (contribution = partial + res/8)
"""

import numpy as np
import ml_dtypes

import concourse.bacc as bacc
import concourse.mybir as mybir
import concourse.tile as tile
from concourse.bass_utils import run_bass_kernel_spmd

AF = mybir.ActivationFunctionType
ALU = mybir.AluOpType
F32 = mybir.dt.float32
F16 = mybir.dt.float16

N_CORES = 8
HID = 4096
S = 1024
NH = 32
HD = 128
NH_L = NH // N_CORES          # 4 local heads
DL = NH_L * HD                # 512 local q/k/v dims
INTER = 11008
IL_T = 11                     # local intermediate k-tiles (padded)
IL = IL_T * 128               # 1408 padded local intermediate
ILR = INTER // N_CORES        # 1376 real local intermediate
KT = HID // 128               # 32 hidden k-tiles
CH = 2                        # seq chunks for attention / MLP rhs
CW = S // CH                  # 512
TB = S // 128                 # 8 seq tiles of 128
EPS = 1e-5

NCH1 = 4                      # seq chunks for AR#1
W1 = S // NCH1
NCH2 = 4                      # hidden-row chunks for AR#2
KH2 = KT // NCH2

_CACHE = {}


def _build(collectives=True, repeat=1):
    nc = bacc.Bacc("TRN2", target_bir_lowering=False, debug=False,
                   num_devices=N_CORES)

    xT = nc.dram_tensor("xT", [HID, S], F16, kind="ExternalInput").ap()
    maskTd = nc.dram_tensor("maskTd", [TB, 128, CW], F16,
                            kind="ExternalInput").ap()
    wqT = nc.dram_tensor("wqT", [HID, DL], F16, kind="ExternalInput").ap()
    wkT = nc.dram_tensor("wkT", [HID, DL], F16, kind="ExternalInput").ap()
    wvT = nc.dram_tensor("wvT", [HID, DL], F16, kind="ExternalInput").ap()
    wosl = nc.dram_tensor("wosl", [KT, 128, DL], F16,
                          kind="ExternalInput").ap()
    wusl = nc.dram_tensor("wusl", [IL_T, 128, KT * 128], F16,
                          kind="ExternalInput").ap()
    wgsl = nc.dram_tensor("wgsl", [IL_T, 128, KT * 128], F16,
                          kind="ExternalInput").ap()
    wdsl = nc.dram_tensor("wdsl", [KT, 128, IL], F16,
                          kind="ExternalInput").ap()
    outT = nc.dram_tensor("outT", [HID, S], F16, kind="ExternalOutput").ap()

    ob1 = [nc.dram_tensor(f"ob1_{i}", [HID, W1], F16).ap()
           for i in range(NCH1)]
    h2sh = [nc.dram_tensor(f"h2sh_{i}", [HID, W1], F16,
                           addr_space="Shared").ap() for i in range(NCH1)]
    s1_d = nc.dram_tensor("s1_d", [S], F32).ap()
    dnb = [nc.dram_tensor(f"dnb{i}", [KH2 * 128, S], F16).ap()
           for i in range(NCH2)]
    dnr = [nc.dram_tensor(f"dnr{i}", [KH2 * 128, S], F16,
                          addr_space="Shared").ap() for i in range(NCH2)]

    rg = [list(range(N_CORES))]

    def all_reduce(dst, srcs, engine):
        if collectives:
            engine.collective_compute(
                "AllReduce", ALU.add, ins=[srcs[:]], outs=[dst[:]],
                replica_groups=rg)
        else:
            engine.dma_start(dst[:], srcs[:])

    with tile.TileContext(nc) as tc:
      for rep in range(repeat):
        P = f"r{rep}_" if repeat > 1 else ""
        with tc.tile_pool(name=P + "const", bufs=1) as const:
            ones = const.tile([128, 128], F16, tag="ones")
            nc.vector.memset(ones[:], 1.0)
            s1 = const.tile([128, S], F32, tag="s1")
            s1t = const.tile([128, TB], F32, tag="s1t")
            epst = const.tile([128, 1], F32, tag="epst")
            nc.vector.memset(epst[:], EPS)

            # ======== Phases 0-2: x load + RMSNorm stats + QKV ========
            with tc.tile_pool(name=P + "qkvo", bufs=1) as qkvo:
                QTt = [qkvo.tile([128, S], F16, tag=f"QT{h}", name=f"QT{h}")
                       for h in range(NH_L)]
                KTt = [qkvo.tile([128, S], F16, tag=f"KT{h}", name=f"KT{h}")
                       for h in range(NH_L)]
                Vt = [qkvo.tile([128, DL], F16, tag=f"V{t}", name=f"V{t}")
                      for t in range(TB)]
                wo_r = [qkvo.tile([128, DL], F16, tag=f"wo{k}", name=f"wo{k}")
                        for k in range(KT)]

                with tc.tile_pool(name=P + "xres", bufs=1) as xres:
                    xt = [xres.tile([128, S], F16, tag=f"x{k}", name=f"x{k}")
                          for k in range(KT)]
                    with (
                        tc.tile_pool(name=P + "p0", bufs=1) as p0,
                        tc.tile_pool(name=P + "p0m", bufs=2) as p0m,
                        tc.tile_pool(name=P + "p0ps", bufs=1,
                                     space="PSUM") as p0ps,
                    ):
                        r2 = [p0ps.tile([128, CW], F32, tag=f"r2_{c}",
                                        name=f"r2_{c}") for c in range(CH)]
                        for k in range(KT):
                            nc.sync.dma_start(xt[k][:],
                                              xT[k * 128:(k + 1) * 128, :])
                            sq = p0.tile([128, S], F16, tag="sq",
                                         name=f"sq{k}")
                            nc.scalar.activation(sq[:], xt[k][:], AF.Square)
                            for c in range(CH):
                                nc.tensor.matmul(
                                    r2[c][:], ones[:],
                                    sq[:, c * CW:(c + 1) * CW],
                                    start=(k == 0), stop=(k == KT - 1))
                        for c in range(CH):
                            ms = p0m.tile([128, CW], F32, tag="ms")
                            nc.scalar.activation(ms[:], r2[c][:], AF.Sqrt,
                                                 bias=epst[:], scale=1.0 / HID)
                            nc.vector.reciprocal(s1[:, c * CW:(c + 1) * CW],
                                                 ms[:])
                    # s1t = s1 transposed down partitions, via a DRAM bounce
                    nc.sync.dma_start(s1_d.rearrange("(o s) -> o s", o=1),
                                      s1[0:1, :])
                    nc.sync.dma_start(s1t[:],
                                      s1_d.rearrange("(t p) -> p t", p=128))

                    # o-proj weight preload (resident; overlaps with q/k/v)
                    for k in range(KT):
                        nc.sync.dma_start(wo_r[k][:], wosl[k, :, :])

                    # q/k passes: weights stream, 8 psum groups, evac x s1
                    for nm, wT, outs in (("q", wqT, QTt), ("k", wkT, KTt)):
                        with (
                            tc.tile_pool(name=P + f"{nm}w", bufs=3) as wp,
                            tc.tile_pool(name=P + f"{nm}ps", bufs=1,
                                         space="PSUM") as ps,
                        ):
                            pt = [ps.tile([128, CW], F32, tag=f"pt{j}",
                                          name=f"pt{j}")
                                  for j in range(NH_L * CH)]
                            for k in range(KT):
                                wt = wp.tile([128, DL], F16, tag="wt")
                                nc.sync.dma_start(
                                    wt[:], wT[k * 128:(k + 1) * 128, :])
                                for h in range(NH_L):
                                    for c in range(CH):
                                        nc.tensor.matmul(
                                            pt[h * CH + c][:],
                                            wt[:, h * 128:(h + 1) * 128],
                                            xt[k][:, c * CW:(c + 1) * CW],
                                            start=(k == 0),
                                            stop=(k == KT - 1))
                            for h in range(NH_L):
                                for c in range(CH):
                                    nc.vector.tensor_mul(
                                        outs[h][:, c * CW:(c + 1) * CW],
                                        pt[h * CH + c][:],
                                        s1[:, c * CW:(c + 1) * CW])

                    # v pass: V[t] rows scaled by s1t column
                    with (
                        tc.tile_pool(name=P + "vw", bufs=3) as wp,
                        tc.tile_pool(name=P + "vps", bufs=1,
                                     space="PSUM") as ps,
                    ):
                        pt = [ps.tile([128, DL], F32, tag=f"pt{t}",
                                      name=f"pt{t}") for t in range(TB)]
                        for k in range(KT):
                            wt = wp.tile([128, DL], F16, tag="wt")
                            nc.sync.dma_start(
                                wt[:], wvT[k * 128:(k + 1) * 128, :])
                            for t in range(TB):
                                nc.tensor.matmul(
                                    pt[t][:], xt[k][:, t * 128:(t + 1) * 128],
                                    wt[:], start=(k == 0), stop=(k == KT - 1))
                        for t in range(TB):
                            nc.vector.tensor_scalar(
                                Vt[t][:], pt[t][:], s1t[:, t:t + 1], None,
                                op0=ALU.mult)

                    # ======== Phase 3: attention ========
                    with tc.tile_pool(name=P + "atp", bufs=1) as atpool:
                      with (
                          tc.tile_pool(name=P + "mask", bufs=1) as mp,
                          tc.tile_pool(name=P + "est", bufs=2) as estp,
                          tc.tile_pool(name=P + "rin", bufs=2) as rinp,
                          tc.tile_pool(name=P + "aps", bufs=1,
                                       space="PSUM") as aps,
                          tc.tile_pool(name=P + "stps", bufs=3,
                                       space="PSUM") as stps,
                      ):
                          mtiles = []
                          for t in range(TB):
                              mt = mp.tile([128, CW], F16, tag=f"m{t}",
                                           name=f"mk{t}")
                              nc.sync.dma_start(mt[:], maskTd[t, :, :])
                              mtiles.append(mt)
                          ATt = [atpool.tile([128, S], F16, tag=f"AT{h}",
                                             name=f"AT{h}")
                                 for h in range(NH_L)]
                          atp = [aps.tile([128, CW], F32, tag=f"atp{j}",
                                          name=f"atp{j}") for j in range(2)]
                          rsp = [aps.tile([128, CW], F32, tag=f"rsp{j}",
                                          name=f"rsp{j}") for j in range(2)]
                          for c in range(CH):
                              for h in range(NH_L):
                                  tbs = list(range(0, (c + 1) * 4))
                                  ets = []
                                  for t in tbs:
                                      stp = stps.tile([128, CW], F32, tag="st")
                                      nc.tensor.matmul(
                                          stp[:],
                                          KTt[h][:, t * 128:(t + 1) * 128],
                                          QTt[h][:, c * CW:(c + 1) * CW],
                                          start=True, stop=True)
                                      et = estp.tile([128, CW], F16,
                                                     tag=f"et{t}",
                                                     name=f"et{t}")
                                      if t >= c * 4:
                                          nc.vector.tensor_add(et[:], stp[:],
                                                               mtiles[t][:])
                                          nc.scalar.activation(et[:], et[:],
                                                               AF.Exp)
                                      else:
                                          nc.scalar.activation(et[:], stp[:],
                                                               AF.Exp)
                                      ets.append(et)
                                  ap_, rp_ = atp[h % 2], rsp[h % 2]
                                  for j, t in enumerate(tbs):
                                      st_, sp_ = (j == 0), (j == len(tbs) - 1)
                                      nc.tensor.matmul(
                                          ap_[:],
                                          Vt[t][:, h * 128:(h + 1) * 128],
                                          ets[j][:], start=st_, stop=sp_)
                                      nc.tensor.matmul(
                                          rp_[:], ones[:], ets[j][:],
                                          start=st_, stop=sp_)
                                  ri = rinp.tile([128, CW], F32, tag="ri")
                                  nc.vector.reciprocal(ri[:], rp_[:])
                                  nc.vector.tensor_mul(
                                      ATt[h][:, c * CW:(c + 1) * CW],
                                      ap_[:], ri[:])

                      # ==== Phase 4: o-proj + x/8 fold -> seq-chunked AR#1 ==
                      with (
                          tc.tile_pool(name=P + "ops", bufs=4,
                                       space="PSUM") as ops,
                          tc.tile_pool(name=P + "oev", bufs=4) as oev,
                      ):
                          for c4 in range(NCH1):
                              sl = slice(c4 * W1, (c4 + 1) * W1)
                              for mh in range(KT):
                                  pt = ops.tile([128, W1], F32, tag="pt")
                                  for h in range(NH_L):
                                      nc.tensor.matmul(
                                          pt[:],
                                          wo_r[mh][:, h * 128:(h + 1) * 128],
                                          ATt[h][:, sl],
                                          start=(h == 0), stop=(h == NH_L - 1))
                                  ev = oev.tile([128, W1], F16, tag="ev")
                                  nc.vector.scalar_tensor_tensor(
                                      ev[:], xt[mh][:, sl], 1.0 / N_CORES,
                                      pt[:], op0=ALU.mult, op1=ALU.add)
                                  nc.sync.dma_start(
                                      ob1[c4][mh * 128:(mh + 1) * 128, :],
                                      ev[:])
                              all_reduce(h2sh[c4], ob1[c4], nc.gpsimd)

            # ===== Phase 5: h2 load + RMSNorm #2 stats (per seq chunk) =====
            with tc.tile_pool(name=P + "h2res", bufs=1) as h2p:
                h2t = [h2p.tile([128, S], F16, tag=f"h2_{k}", name=f"h2_{k}")
                       for k in range(KT)]
                s2 = h2p.tile([128, S], F32, tag="s2", name="s2")
                with (
                    tc.tile_pool(name=P + "p5", bufs=3) as p5,
                    tc.tile_pool(name=P + "p5m", bufs=2) as p5m,
                    tc.tile_pool(name=P + "p5ps", bufs=2, space="PSUM") as p5ps,
                ):
                    for c4 in range(NCH1):
                        sl = slice(c4 * W1, (c4 + 1) * W1)
                        r2 = p5ps.tile([128, W1], F32, tag="r2c",
                                       name=f"r2c{c4}")
                        for k in range(KT):
                            nc.sync.dma_start(
                                h2t[k][:, sl],
                                h2sh[c4][k * 128:(k + 1) * 128, :])
                            sq = p5.tile([128, W1], F16, tag="sq",
                                         name=f"sq5_{k}")
                            nc.scalar.activation(sq[:], h2t[k][:, sl],
                                                 AF.Square)
                            nc.tensor.matmul(r2[:], ones[:], sq[:],
                                             start=(k == 0),
                                             stop=(k == KT - 1))
                        ms = p5m.tile([128, W1], F32, tag="ms")
                        nc.scalar.activation(ms[:], r2[:], AF.Sqrt,
                                             bias=epst[:], scale=1.0 / HID)
                        nc.vector.reciprocal(s2[:, sl], ms[:])

                # ===== Phase 6: up/gate + silu-mul (s2 folded at evac) =====
                with tc.tile_pool(name=P + "mres", bufs=1) as mres:
                    m_t = [mres.tile([128, S], F16, tag=f"m{i}",
                                     name=f"mres{i}") for i in range(IL_T)]
                    with (
                        tc.tile_pool(name=P + "ugw", bufs=2) as ugw,
                        tc.tile_pool(name=P + "ugps", bufs=1,
                                     space="PSUM") as ugps,
                        tc.tile_pool(name=P + "ugt", bufs=2) as ugt,
                    ):
                        for d in range(IL_T):
                            slabs = {}
                            for nm, wsl in (("u", wusl), ("g", wgsl)):
                                slb = ugw.tile([128, KT * 128], F16,
                                               tag=f"slab{nm}",
                                               name=f"slab_{nm}{d}")
                                nc.sync.dma_start(slb[:], wsl[d, :, :])
                                slabs[nm] = slb
                            CH67, CW67 = 4, S // 4
                            pts = {}
                            for nm in ("u", "g"):
                                for c in range(CH67):
                                    pt = ugps.tile([128, CW67], F32,
                                                   tag=f"pt{nm}{c}",
                                                   name=f"pt{nm}{c}")
                                    for k in range(KT):
                                        nc.tensor.matmul(
                                            pt[:],
                                            slabs[nm][:,
                                                      k * 128:(k + 1) * 128],
                                            h2t[k][:,
                                                   c * CW67:(c + 1) * CW67],
                                            start=(k == 0),
                                            stop=(k == KT - 1))
                                    pts[(nm, c)] = pt
                            for c in range(CH67):
                                s2c = s2[:, c * CW67:(c + 1) * CW67]
                                un = ugt.tile([128, CW67], F32, tag="un")
                                nc.vector.tensor_mul(un[:], pts[("u", c)][:],
                                                     s2c)
                                sil = ugt.tile([128, CW67], F32, tag="sil")
                                nc.scalar.activation(sil[:], un[:], AF.Silu)
                                gn = ugt.tile([128, CW67], F32, tag="gn")
                                nc.vector.tensor_mul(gn[:], pts[("g", c)][:],
                                                     s2c)
                                nc.vector.tensor_mul(
                                    m_t[d][:, c * CW67:(c + 1) * CW67],
                                    sil[:], gn[:])

                    # ===== Phase 7: down-proj + h2/8 fold -> chunked AR#2 ==
                    with (
                        tc.tile_pool(name=P + "dw", bufs=2) as dwp,
                        tc.tile_pool(name=P + "dps", bufs=2,
                                     space="PSUM") as dps,
                        tc.tile_pool(name=P + "dev", bufs=2) as dev,
                    ):
                        for half in range(NCH2):
                            for mh0 in range(KH2):
                                mh = half * KH2 + mh0
                                slb = dwp.tile([128, IL], F16, tag="dw")
                                nc.sync.dma_start(slb[:], wdsl[mh, :, :])
                                for c in range(CH):
                                    pt = dps.tile([128, CW], F32, tag="pt")
                                    for i in range(IL_T):
                                        nc.tensor.matmul(
                                            pt[:],
                                            slb[:, i * 128:(i + 1) * 128],
                                            m_t[i][:, c * CW:(c + 1) * CW],
                                            start=(i == 0),
                                            stop=(i == IL_T - 1))
                                    ev = dev.tile([128, CW], F16, tag="ev")
                                    nc.vector.scalar_tensor_tensor(
                                        ev[:],
                                        h2t[mh][:, c * CW:(c + 1) * CW],
                                        1.0 / N_CORES, pt[:], op0=ALU.mult,
                                        op1=ALU.add)
                                    nc.sync.dma_start(
                                        dnb[half][mh0 * 128:(mh0 + 1) * 128,
                                                  c * CW:(c + 1) * CW],
                                        ev[:])
                            all_reduce(dnr[half], dnb[half], nc.gpsimd)
            for half in range(NCH2):
                nc.sync.dma_start(outT[half * KH2 * 128:
                                       (half + 1) * KH2 * 128, :],
                                  dnr[half][:])

    nc.compile()
    return nc


def _host_shard(hidden_states, mask, wq, wk, wv, wo, w_gate, w_up, w_down,
                g_in, g_post):
    f16 = np.float16
    x = np.asarray(hidden_states, dtype=np.float32).reshape(S, HID)
    xT = np.ascontiguousarray(x.T).astype(f16)
    maskT = np.ascontiguousarray(
        np.clip(np.asarray(mask, dtype=np.float32), -30000.0, 0.0)
        .reshape(S, S).T)
    maskTd = np.empty((TB, 128, CW), f16)
    for t in range(TB):
        c = t // (TB // CH)
        maskTd[t] = maskT[t * 128:(t + 1) * 128, c * CW:(c + 1) * CW]
    g_in = np.asarray(g_in, dtype=np.float32)
    g_post = np.asarray(g_post, dtype=np.float32)
    sc = np.float32(HD ** -0.5)

    in_maps = []
    for i in range(N_CORES):
        r0, r1 = i * DL, (i + 1) * DL
        i0, i1 = i * ILR, (i + 1) * ILR
        wqT = np.ascontiguousarray(wq[r0:r1].T * (g_in[:, None] * sc))
        wkT = np.ascontiguousarray(wk[r0:r1].T * g_in[:, None])
        wvT = np.ascontiguousarray(wv[r0:r1].T * g_in[:, None])
        # o slabs: lhsT[p, h*128+j] = woT[h*128+p, mh*128+j]
        #   (p = contraction over local attn dims, j = output hidden row)
        woT = np.ascontiguousarray(wo[:, r0:r1].T)
        wosl = np.ascontiguousarray(
            woT.reshape(NH_L, 128, KT, 128).transpose(2, 1, 0, 3)
            .reshape(KT, 128, DL)).astype(f16)
        # up/gate slabs: w[d, p, k*128+j] = wT[k*128+p, d*128+j]
        #   where wT = w_up[i0:i1].T * g_post (HID x IL, zero-padded)
        wuT = np.zeros((HID, IL), np.float32)
        wuT[:, :ILR] = w_up[i0:i1].T * g_post[:, None]
        wgT = np.zeros((HID, IL), np.float32)
        wgT[:, :ILR] = w_gate[i0:i1].T * g_post[:, None]
        wusl = np.ascontiguousarray(
            wuT.reshape(KT, 128, IL_T, 128).transpose(2, 1, 0, 3)
            .reshape(IL_T, 128, KT * 128)).astype(f16)
        wgsl = np.ascontiguousarray(
            wgT.reshape(KT, 128, IL_T, 128).transpose(2, 1, 0, 3)
            .reshape(IL_T, 128, KT * 128)).astype(f16)
        # down slabs: wdsl[mh, p, i*128+j] = wdT[i*128+p, mh*128+j]
        #   where wdT = w_down[:, i0:i1].T (IL x HID, zero-padded)
        wdT = np.zeros((IL, HID), np.float32)
        wdT[:ILR] = w_down[:, i0:i1].T
        wdsl = np.ascontiguousarray(
            wdT.reshape(IL_T, 128, KT, 128).transpose(2, 1, 0, 3)
            .reshape(KT, 128, IL)).astype(f16)
        in_maps.append({
            "xT": xT, "maskTd": maskTd,
            "wqT": wqT.astype(f16), "wkT": wkT.astype(f16),
            "wvT": wvT.astype(f16), "wosl": wosl,
            "wusl": wusl, "wgsl": wgsl, "wdsl": wdsl,
        })
    return in_maps


def _get_nc(repeat=1):
    key = ("nc", repeat)
    if key not in _CACHE:
        _CACHE[key] = _build(repeat=repeat)
    return _CACHE[key]


def kernel(**inputs):
    nc = _get_nc()
    in_maps = _host_shard(**{k: np.asarray(v) for k, v in inputs.items()})
    res = run_bass_kernel_spmd(nc, in_maps, list(range(N_CORES)))
    outT = res.results[0]["outT"].astype(np.float32)
    return np.ascontiguousarray(outT.T).reshape(1, S, HID)


def _make_runner(repeat=1, **inputs):
    """Build the compiled sharded callable + device-resident inputs once.
    Returns run() -> (wall_ns, outs)."""
    import time
    import jax
    from jax.sharding import Mesh, PartitionSpec
    from jax.experimental.shard_map import shard_map
    from concourse import bass2jax

    nc = _get_nc(repeat)
    in_maps = _host_shard(**{k: np.asarray(v) for k, v in inputs.items()})
    bass2jax.install_neuronx_cc_hook()

    partition_name = (nc.partition_id_tensor.name
                      if nc.partition_id_tensor else None)
    in_names, out_names, out_avals, zero_outs = [], [], [], []
    for alloc in nc.m.functions[0].allocations:
        if not isinstance(alloc, mybir.MemoryLocationSet):
            continue
        name = alloc.memorylocations[0].name
        if alloc.kind == "ExternalInput":
            if name != partition_name:
                in_names.append(name)
        elif alloc.kind == "ExternalOutput":
            out_names.append(name)
            shape = tuple(alloc.tensor_shape)
            dtype = mybir.dt.np(alloc.dtype)
            out_avals.append(jax.core.ShapedArray(shape, dtype))
            zero_outs.append(np.zeros(shape, dtype))
    n_params = len(in_names)
    all_in = list(in_names) + list(out_names)
    if partition_name is not None:
        all_in.append(partition_name)

    def _body(*args):
        operands = list(args)
        if partition_name is not None:
            operands.append(bass2jax.partition_id_tensor())
        outs = bass2jax._bass_exec_p.bind(
            *operands,
            out_avals=tuple(out_avals), in_names=tuple(all_in),
            out_names=tuple(out_names), lowering_input_output_aliases=(),
            sim_require_finite=True, sim_require_nnan=True, nc=nc)
        return tuple(outs)

    devices = jax.devices()[:N_CORES]
    mesh = Mesh(np.asarray(devices), ("core",))
    n_outs = len(out_names)
    in_specs = (PartitionSpec("core"),) * (n_params + n_outs)
    out_specs = (PartitionSpec("core"),) * n_outs
    fn = jax.jit(shard_map(_body, mesh=mesh, in_specs=in_specs,
                           out_specs=out_specs, check_rep=False))
    concat_in = [np.concatenate([np.asarray(in_maps[c][nm])
                                 for c in range(N_CORES)], axis=0)
                 for nm in in_names]
    concat_zeros = [np.zeros((N_CORES * z.shape[0], *z.shape[1:]), z.dtype)
                    for z in zero_outs]
    sharding = jax.sharding.NamedSharding(mesh, PartitionSpec("core"))
    dev_in = [jax.device_put(a, sharding) for a in concat_in]
    dev_zero = [jax.device_put(a, sharding) for a in concat_zeros]

    outs = fn(*dev_in, *dev_zero)          # warm-up / compile
    jax.block_until_ready(outs)

    def run():
        t0 = time.perf_counter_ns()
        o = fn(*dev_in, *dev_zero)
        jax.block_until_ready(o)
        return time.perf_counter_ns() - t0, o

    def unpack(o):
        return {nm: np.asarray(o[i]).reshape(N_CORES, *out_avals[i].shape)[0]
                for i, nm in enumerate(out_names)}

    return run, unpack

